# revision 11
# baseline (speedup 1.0000x reference)
"""TRN2 Bass kernel: 100 sequential Linear layers (y = x @ W^T + b).

The network has no activations, so it is one affine map: y = x @ M + c with
M = W1^T @ ... @ W100^T. Each core computes a 12-13 layer segment product
(G_k = W_hi...W_lo plus the bias column u_k, chained 512x514 fp32r matmuls),
the 8 [G_k|u_k] blocks are AllGathered in bf16 (one collective: the CC
firmware serializes collectives and each costs ~40us latency regardless of
payload), every core folds them to (M, c) redundantly (bf16, fp32 PSUM),
and applies y_shard = x_shard @ M + c to its 2048-row batch shard.

Perf notes: DMA triggers cost ~600ns of queue time, so bulk loads are one
contiguous [128, X] DMA from host pre-shuffled layouts, and the post-AG
readback alternates between the sync and scalar queues. LDWEIGHTS is
emitted per matmul (~100-150ns, serial with the stream), so the apply runs
d-outer in bf16 with one stationary per 4 batch chunks. A tiny dummy
AllReduce at t=0 absorbs the collective firmware's startup cost under the
chain. The chain's last-step PSUM copies emit bf16 directly into the
AllGather staging tiles.
"""
import os
import numpy as np
import ml_dtypes

import concourse.bacc as bacc
import concourse.mybir as mybir
import concourse.tile as tile
import concourse.bass_utils as bass_utils
from concourse.bass_utils import run_bass_kernel_spmd

f32 = mybir.dt.float32
f32r = mybir.dt.float32r
bf16 = mybir.dt.bfloat16

N_CORES = 8
N_LAYERS = 100
D = 512
BATCH = 16384
B = BATCH // N_CORES     # 2048 rows per core
NSTEPS = 12              # uniform chain steps per core (after the init layer)
SW = D + 2               # state width: 512 G cols + bias col + even-pad col
ND = 4                   # 128-row tiles of the 512 dim
NB = B // 512            # batch chunks per core
SEG_LENS = [13, 13, 13, 13, 12, 12, 12, 12]

LAST_EXEC_TIME_NS = None
LAST_RESULTS = None

# Artifact upload to the fish bucket is unreachable from this container.
bass_utils.upload_artifacts = lambda d: d

_NC_CACHE = {}


def _build_nc():
    nc = bacc.Bacc("TRN2", target_bir_lowering=False, debug=False,
                   num_devices=N_CORES)
    # all inputs are host pre-shuffled into [128, ...] partition-major blocks
    xT = nc.declare_dram_parameter("xT", [128, ND * B], bf16, isOutput=False)
    WT = nc.declare_dram_parameter("WT", [NSTEPS, 128, ND * D], bf16,
                                   isOutput=False)
    Winit = nc.declare_dram_parameter("Winit", [128, ND * SW], bf16,
                                      isOutput=False)
    bRow = nc.declare_dram_parameter("bRow", [1, NSTEPS * D], bf16,
                                     isOutput=False)
    onehotB = nc.declare_dram_parameter("onehotB", [1, 2], bf16,
                                        isOutput=False)
    onesRow = nc.declare_dram_parameter("onesRow", [1, D], bf16, isOutput=False)
    eyeB = nc.declare_dram_parameter("eyeB", [128, ND * D], bf16,
                                     isOutput=False)
    yT = nc.declare_dram_parameter("yT", [128, ND * B], f32r, isOutput=True)

    with tile.TileContext(nc) as tc:
        with tc.tile_pool(name="consts", bufs=1) as cpool, \
             tc.tile_pool(name="xp", bufs=1) as xpool, \
             tc.tile_pool(name="dram", bufs=1, space="DRAM") as dram:

            # ---- early tiny collective to absorb CC startup cost ----
            # AllGather (N wire, ~half an AllReduce's mesh phases) over an
            # uninitialized byte buffer (the output is never read), so the
            # trigger fires right after the gpsimd preamble and the CC
            # firmware frees up before the real AllGather needs it
            d_in = dram.tile([1, 64], f32, name="d_in")
            d_out = dram.tile([N_CORES, 64], f32, name="d_out")
            nc.gpsimd.collective_compute(
                "AllGather", mybir.AluOpType.bypass,
                replica_groups=[list(range(N_CORES))],
                ins=[d_in.opt()], outs=[d_out.opt()],
            )

            bRow_sb = cpool.tile([1, NSTEPS * D], bf16, name="bRow_sb")
            onehot_sb = cpool.tile([1, 2], bf16, name="onehot_sb")
            ones_sb = cpool.tile([1, D], bf16, name="ones_sb")
            eye_sb = cpool.tile([128, ND * D], bf16, name="eye_sb")
            nc.gpsimd.dma_start(out=bRow_sb, in_=bRow[:, :])
            nc.gpsimd.dma_start(out=onehot_sb, in_=onehotB[:, :])
            nc.gpsimd.dma_start(out=ones_sb, in_=onesRow[:, :])
            nc.gpsimd.dma_start(out=eye_sb, in_=eyeB[:, :])
            xsb = xpool.tile([128, ND * B], bf16, name="xsb")

            in_b = dram.tile([D, SW], bf16, name="in_b")
            out_b = dram.tile([N_CORES * D, SW], bf16, name="out_b",
                              addr_space="Shared")

            # ---- chain: 12 fp32r steps; last step emits bf16 staging ----
            with tc.tile_pool(name="si", bufs=1) as si_pool, \
                 tc.tile_pool(name="S", bufs=2) as S_pool, \
                 tc.tile_pool(name="sbf", bufs=1) as sbf_pool, \
                 tc.tile_pool(name="wp", bufs=6) as w_pool, \
                 tc.tile_pool(name="cps", bufs=4, space="PSUM") as cpsa, \
                 tc.tile_pool(name="cpsb", bufs=4, space="PSUM") as cpsb:
                Sinit = si_pool.tile([128, ND * SW], bf16, name="Sinit")
                nc.scalar.dma_start(out=Sinit, in_=Winit[:, :])

                S = None
                for step in range(NSTEPS):
                    Wl = w_pool.tile([128, ND * D], bf16, name=f"W_{step}",
                                     tag="W")
                    nc.sync.dma_start(out=Wl, in_=WT[step, :, :])
                    if step == NSTEPS - 1:
                        # x is only needed by the apply; issue after the Ws
                        nc.sync.dma_start(out=xsb, in_=xT[:, :])
                    last = (step == NSTEPS - 1)
                    if last:
                        Snew = [sbf_pool.tile([128, SW], bf16,
                                              name=f"Sb_{j}")
                                for j in range(ND)]
                    else:
                        Snew = [S_pool.tile([128, SW], bf16,
                                            name=f"S{step + 1}_{j}",
                                            tag=f"S{j}")
                                for j in range(ND)]

                    def s_ap(d, c0, c1):
                        if S is None:
                            return Sinit[:, d * SW + c0:d * SW + c1]
                        return S[d][:, c0:c1]

                    for j in range(ND):
                        psA = cpsa.tile([128, 512], f32, name=f"psA_{step}_{j}",
                                       tag="psa")
                        psB = cpsb.tile([128, 2], f32, name=f"psB_{step}_{j}",
                                       tag="psb")
                        for d in range(ND):
                            w_ap = Wl[:, d * D + j * 128:d * D + (j + 1) * 128]
                            nc.tensor.matmul(
                                psA, w_ap, s_ap(d, 0, 512),
                                start=(d == 0), stop=(d == ND - 1))
                            nc.tensor.matmul(
                                psB, w_ap, s_ap(d, 512, 514),
                                start=(d == 0), stop=False)
                        nc.tensor.matmul(
                            psB,
                            bRow_sb[0:1, step * D + j * 128:
                                    step * D + (j + 1) * 128],
                            onehot_sb,
                            start=False, stop=True)
                        if j % 2 == 0:
                            nc.vector.tensor_copy(Snew[j][:, 0:512], psA)
                            nc.scalar.copy(out=Snew[j][:, 512:514], in_=psB)
                        else:
                            nc.scalar.copy(out=Snew[j][:, 0:512], in_=psA)
                            nc.vector.tensor_copy(Snew[j][:, 512:514], psB)
                        if last:
                            eng = nc.sync if j % 2 == 0 else nc.scalar
                            eng.dma_start(
                                out=in_b[j * 128:(j + 1) * 128, :],
                                in_=Snew[j])
                    S = Snew

                # ---- AllGather (the only exposed collective) ----
                nc.gpsimd.collective_compute(
                    "AllGather", mybir.AluOpType.bypass,
                    replica_groups=[list(range(N_CORES))],
                    ins=[in_b.opt()], outs=[out_b.opt()],
                )

            # ---- readback + fold + apply ----
            with tc.tile_pool(name="g8", bufs=1) as g8_pool, \
                 tc.tile_pool(name="zb", bufs=2) as zb_pool, \
                 tc.tile_pool(name="mp", bufs=1) as m_pool, \
                 tc.tile_pool(name="fps", bufs=7, space="PSUM") as fps, \
                 tc.tile_pool(name="ups", bufs=1, space="PSUM") as ups, \
                 tc.tile_pool(name="yo", bufs=2) as yo_pool:
                G = {}
                # k = 7 first (the fold consumes it first); alternate queues
                # so trigger issue time does not throttle the fold
                qs = [nc.sync, nc.scalar, nc.gpsimd]
                for k in range(N_CORES - 1, -1, -1):
                    for d in range(ND):
                        g = g8_pool.tile([128, SW], bf16, name=f"G{k}_{d}")
                        qs[(k * ND + d) % 3].dma_start(
                            out=g,
                            in_=out_b[k * D + d * 128:k * D + (d + 1) * 128, :])
                        G[(k, d)] = g

                # ---- fold to (M, c_row) ----
                psu = ups.tile([1, D], f32, name="psu")
                Zcur = None  # k=7 uses the bf16 identity as rhs
                M_tiles = None
                for k in range(N_CORES - 1, -1, -1):
                    lastf = (k == 0)
                    if lastf:
                        Znew = [m_pool.tile([128, D], bf16, name=f"M_{j}")
                                for j in range(ND)]
                        M_tiles = Znew
                    else:
                        Znew = [zb_pool.tile([128, D], bf16,
                                             name=f"Z{k}_{j}", tag=f"Z{j}")
                                for j in range(ND)]

                    def z_ap(d):
                        if Zcur is None:
                            return eye_sb[:, d * D:(d + 1) * D]
                        return Zcur[d]

                    psZ = [fps.tile([128, D], f32, name=f"psZ_{k}_{j}",
                                    tag="psf") for j in range(ND)]
                    # d-outer: consume one gathered G tile at a time so the
                    # first fold step overlaps the post-AG readback
                    for d in range(ND):
                        for j in range(ND):
                            nc.tensor.matmul(
                                psZ[j],
                                G[(k, d)][:, j * 128:(j + 1) * 128],
                                z_ap(d),
                                start=(d == 0), stop=(d == ND - 1))
                    for j in range(ND):
                        if j % 2 == 0:
                            nc.vector.tensor_copy(Znew[j], psZ[j])
                        else:
                            nc.scalar.copy(out=Znew[j], in_=psZ[j])
                    for d in range(ND):
                        nc.tensor.matmul(
                            psu,
                            G[(k, d)][:, D:D + 1],
                            z_ap(d),
                            start=(k == N_CORES - 1 and d == 0),
                            stop=(k == 0 and d == ND - 1))
                    Zcur = Znew
                c_row = cpool.tile([1, D], bf16, name="c_row")
                nc.vector.tensor_copy(c_row, psu)
                # transpose c_row into per-partition bias columns [128,1]x4
                cT = []
                for j in range(ND):
                    pst = fps.tile([128, 512], f32, name=f"pst_{j}",
                                   tag="psf")
                    nc.tensor.matmul(
                        pst[:, 0:1],
                        c_row[0:1, j * 128:(j + 1) * 128],
                        ones_sb[0:1, 0:1],
                        start=True, stop=True)
                    ct = cpool.tile([128, 1], f32, name=f"cT_{j}")
                    nc.vector.tensor_copy(ct, pst[:, 0:1])
                    cT.append(ct)

                # ---- apply y^T = M^T x^T + c, d-outer ----
                for j in range(ND):
                    psY = [fps.tile([128, 512], f32, name=f"psY_{n}_{j}",
                                    tag="psf") for n in range(NB)]
                    for d in range(ND):
                        for n in range(NB):
                            nc.tensor.matmul(
                                psY[n],
                                M_tiles[d][:, j * 128:(j + 1) * 128],
                                xsb[:, d * B + n * 512:d * B + (n + 1) * 512],
                                start=(d == 0), stop=(d == ND - 1))
                    yo = yo_pool.tile([128, B], f32r, name=f"yo_{j}", tag="yo")
                    for n in range(NB):
                        if n % 2 == 0:
                            nc.vector.tensor_scalar_add(
                                out=yo[:, n * 512:(n + 1) * 512],
                                in0=psY[n], scalar1=cT[j])
                        else:
                            nc.scalar.add(out=yo[:, n * 512:(n + 1) * 512],
                                          in_=psY[n], add=cT[j])
                        nc.sync.dma_start(
                            out=yT[:, j * B + n * 512:j * B + (n + 1) * 512],
                            in_=yo[:, n * 512:(n + 1) * 512])

    nc.compile()
    return nc


def _get_nc():
    key = "default"
    if key not in _NC_CACHE:
        _NC_CACHE[key] = _build_nc()
    return _NC_CACHE[key]


def _segment_bounds():
    bounds = []
    lo = 0
    for ln in SEG_LENS:
        bounds.append((lo, lo + ln))
        lo += ln
    assert lo == N_LAYERS
    return bounds


def _pm(a):
    """[512, X] -> partition-major [128, 4*X] (d-tile blocks side by side)."""
    x = a.shape[1]
    return np.ascontiguousarray(
        a.reshape(ND, 128, x).transpose(1, 0, 2).reshape(128, ND * x))


def kernel(x: np.ndarray, Ws: np.ndarray, bs: np.ndarray) -> np.ndarray:
    global LAST_EXEC_TIME_NS, LAST_RESULTS
    x = np.ascontiguousarray(np.asarray(x, dtype=np.float32))
    Ws = np.ascontiguousarray(np.asarray(Ws, dtype=np.float32))
    bs = np.ascontiguousarray(np.asarray(bs, dtype=np.float32))

    onehot = np.zeros((1, 2), dtype=ml_dtypes.bfloat16)
    onehot[0, 0] = 1.0
    ones_row = np.ones((1, D), dtype=ml_dtypes.bfloat16)
    eye_b = _pm(np.eye(D, dtype=np.float32)).astype(ml_dtypes.bfloat16)

    in_maps = []
    for i, (lo, hi) in enumerate(_segment_bounds()):
        if hi - lo == NSTEPS + 1:
            winit = np.concatenate(
                [Ws[lo], bs[lo][:, None],
                 np.zeros((D, 1), dtype=np.float32)], axis=1)
            steps = list(range(lo + 1, hi))
        else:
            winit = np.concatenate(
                [np.eye(D, dtype=np.float32),
                 np.zeros((D, 2), dtype=np.float32)], axis=1)
            steps = list(range(lo, hi))
        assert len(steps) == NSTEPS
        WTp = np.stack([_pm(np.ascontiguousarray(Ws[l].T)) for l in steps],
                       axis=0).astype(ml_dtypes.bfloat16)
        brow = np.ascontiguousarray(
            np.stack([bs[l] for l in steps], axis=0).reshape(
                1, NSTEPS * D).astype(ml_dtypes.bfloat16))
        shard = _pm(np.ascontiguousarray(
            x[i * B:(i + 1) * B, :].T)).astype(ml_dtypes.bfloat16)
        in_maps.append({
            "xT": shard,
            "WT": np.ascontiguousarray(WTp),
            "Winit": _pm(winit).astype(ml_dtypes.bfloat16),
            "bRow": brow,
            "onehotB": onehot,
            "onesRow": ones_row,
            "eyeB": eye_b,
        })

    nc = _get_nc()
    trace = os.environ.get("BASS_KERNEL_TRACE", "0") == "1"
    res = run_bass_kernel_spmd(nc, in_maps, list(range(N_CORES)), trace=trace)
    LAST_EXEC_TIME_NS = res.exec_time_ns
    LAST_RESULTS = res

    outs = []
    for i in range(N_CORES):
        yp = res.results[i]["yT"]  # [128, 4*2048]: yp[p, j*B+n] = y[n, j*128+p]
        y = yp.reshape(128, ND, B).transpose(2, 1, 0).reshape(B, D)
        outs.append(y)
    y = np.concatenate(outs, axis=0)
    return np.ascontiguousarray(y.astype(np.float32))


# revision 12
# speedup vs baseline: 1.2599x; 1.2599x over previous
"""TRN2 Bass kernel: 100 sequential Linear layers (y = x @ W^T + b).

The network has no activations, so it is one affine map: y = x @ M + c with
M = W1^T @ ... @ W100^T. Each core computes a 12-13 layer segment product
(G_k = W_hi...W_lo plus the bias column u_k, chained 512x514 fp32r matmuls),
the 8 [G_k|u_k] blocks are AllGathered in bf16 (one collective: the CC
firmware serializes collectives and each costs ~40us latency regardless of
payload), every core folds them to (M, c) redundantly (bf16, fp32 PSUM),
and applies y_shard = x_shard @ M + c to its 2048-row batch shard.

Perf notes: DMA triggers cost ~600ns of queue time, so bulk loads are one
contiguous [128, X] DMA from host pre-shuffled layouts, and the post-AG
readback alternates between the sync and scalar queues. LDWEIGHTS is
emitted per matmul (~100-150ns, serial with the stream), so the apply runs
d-outer in bf16 with one stationary per 4 batch chunks. A tiny dummy
AllReduce at t=0 absorbs the collective firmware's startup cost under the
chain. The chain's last-step PSUM copies emit bf16 directly into the
AllGather staging tiles.
"""
import os
import numpy as np
import ml_dtypes

import concourse.bacc as bacc
import concourse.mybir as mybir
import concourse.tile as tile
import concourse.bass_utils as bass_utils
from concourse.bass_utils import run_bass_kernel_spmd

f32 = mybir.dt.float32
f32r = mybir.dt.float32r
bf16 = mybir.dt.bfloat16

N_CORES = 8
N_LAYERS = 100
D = 512
BATCH = 16384
B = BATCH // N_CORES     # 2048 rows per core
NSTEPS = 12              # uniform chain steps per core (after the init layer)
SW = D + 2               # state width: 512 G cols + bias col + even-pad col
ND = 4                   # 128-row tiles of the 512 dim
NB = B // 512            # batch chunks per core
SEG_LENS = [13, 13, 13, 13, 12, 12, 12, 12]

LAST_EXEC_TIME_NS = None
LAST_RESULTS = None

# Artifact upload to the fish bucket is unreachable from this container.
bass_utils.upload_artifacts = lambda d: d

_NC_CACHE = {}


def _build_nc():
    nc = bacc.Bacc("TRN2", target_bir_lowering=False, debug=False,
                   num_devices=N_CORES)
    # all inputs are host pre-shuffled into [128, ...] partition-major blocks
    xT = nc.declare_dram_parameter("xT", [128, ND * B], bf16, isOutput=False)
    WT = nc.declare_dram_parameter("WT", [NSTEPS, 128, ND * D], bf16,
                                   isOutput=False)
    Winit = nc.declare_dram_parameter("Winit", [128, ND * SW], bf16,
                                      isOutput=False)
    bRow = nc.declare_dram_parameter("bRow", [1, NSTEPS * D], bf16,
                                     isOutput=False)
    onehotB = nc.declare_dram_parameter("onehotB", [1, 2], bf16,
                                        isOutput=False)
    onesRow = nc.declare_dram_parameter("onesRow", [1, D], bf16, isOutput=False)
    eyeB = nc.declare_dram_parameter("eyeB", [128, ND * D], bf16,
                                     isOutput=False)
    yT = nc.declare_dram_parameter("yT", [128, ND * B], f32r, isOutput=True)

    with tile.TileContext(nc) as tc:
        with tc.tile_pool(name="consts", bufs=1) as cpool, \
             tc.tile_pool(name="xp", bufs=1) as xpool, \
             tc.tile_pool(name="dram", bufs=1, space="DRAM") as dram:

            # ---- early tiny collective to absorb CC startup cost ----
            # AllGather (N wire, ~half an AllReduce's mesh phases) over an
            # uninitialized byte buffer (the output is never read), so the
            # trigger fires right after the gpsimd preamble and the CC
            # firmware frees up before the real AllGather needs it
            d_in = dram.tile([1, 64], f32, name="d_in")
            d_out = dram.tile([N_CORES, 64], f32, name="d_out")
            nc.gpsimd.collective_compute(
                "AllGather", mybir.AluOpType.bypass,
                replica_groups=[list(range(N_CORES))],
                ins=[d_in.opt()], outs=[d_out.opt()],
            )

            bRow_sb = cpool.tile([1, NSTEPS * D], bf16, name="bRow_sb")
            onehot_sb = cpool.tile([1, 2], bf16, name="onehot_sb")
            ones_sb = cpool.tile([1, D], bf16, name="ones_sb")
            eye_sb = cpool.tile([128, ND * D], bf16, name="eye_sb")
            nc.gpsimd.dma_start(out=bRow_sb, in_=bRow[:, :])
            nc.gpsimd.dma_start(out=onehot_sb, in_=onehotB[:, :])
            nc.gpsimd.dma_start(out=ones_sb, in_=onesRow[:, :])
            nc.gpsimd.dma_start(out=eye_sb, in_=eyeB[:, :])
            xsb = xpool.tile([128, ND * B], bf16, name="xsb")

            in_b = dram.tile([D, SW], bf16, name="in_b")
            out_b = dram.tile([N_CORES * D, SW], bf16, name="out_b",
                              addr_space="Shared")

            # ---- chain: 12 fp32r steps; last step emits bf16 staging ----
            with tc.tile_pool(name="si", bufs=1) as si_pool, \
                 tc.tile_pool(name="S", bufs=2) as S_pool, \
                 tc.tile_pool(name="sbf", bufs=1) as sbf_pool, \
                 tc.tile_pool(name="wp", bufs=6) as w_pool, \
                 tc.tile_pool(name="cps", bufs=4, space="PSUM") as cpsa, \
                 tc.tile_pool(name="cpsb", bufs=4, space="PSUM") as cpsb:
                Sinit = si_pool.tile([128, ND * SW], bf16, name="Sinit")
                nc.scalar.dma_start(out=Sinit, in_=Winit[:, :])

                S = None
                for step in range(NSTEPS):
                    Wl = w_pool.tile([128, ND * D], bf16, name=f"W_{step}",
                                     tag="W")
                    nc.sync.dma_start(out=Wl, in_=WT[step, :, :])
                    if step == NSTEPS - 1:
                        # x is only needed by the apply; issue after the Ws
                        nc.sync.dma_start(out=xsb, in_=xT[:, :])
                    last = (step == NSTEPS - 1)
                    if last:
                        Snew = [sbf_pool.tile([128, SW], bf16,
                                              name=f"Sb_{j}")
                                for j in range(ND)]
                    else:
                        Snew = [S_pool.tile([128, SW], bf16,
                                            name=f"S{step + 1}_{j}",
                                            tag=f"S{j}")
                                for j in range(ND)]

                    def s_ap(d, c0, c1):
                        if S is None:
                            return Sinit[:, d * SW + c0:d * SW + c1]
                        return S[d][:, c0:c1]

                    for j in range(ND):
                        psA = cpsa.tile([128, 512], f32, name=f"psA_{step}_{j}",
                                       tag="psa")
                        psB = cpsb.tile([128, 2], f32, name=f"psB_{step}_{j}",
                                       tag="psb")
                        for d in range(ND):
                            w_ap = Wl[:, d * D + j * 128:d * D + (j + 1) * 128]
                            nc.tensor.matmul(
                                psA, w_ap, s_ap(d, 0, 512),
                                start=(d == 0), stop=(d == ND - 1))
                            nc.tensor.matmul(
                                psB, w_ap, s_ap(d, 512, 514),
                                start=(d == 0), stop=False)
                        nc.tensor.matmul(
                            psB,
                            bRow_sb[0:1, step * D + j * 128:
                                    step * D + (j + 1) * 128],
                            onehot_sb,
                            start=False, stop=True)
                        if j % 2 == 0:
                            nc.vector.tensor_copy(Snew[j][:, 0:512], psA)
                            nc.scalar.copy(out=Snew[j][:, 512:514], in_=psB)
                        else:
                            nc.scalar.copy(out=Snew[j][:, 0:512], in_=psA)
                            nc.vector.tensor_copy(Snew[j][:, 512:514], psB)
                        if last:
                            eng = nc.sync if j % 2 == 0 else nc.scalar
                            eng.dma_start(
                                out=in_b[j * 128:(j + 1) * 128, :],
                                in_=Snew[j])
                    S = Snew

                # ---- AllGather (the only exposed collective) ----
                nc.gpsimd.collective_compute(
                    "AllGather", mybir.AluOpType.bypass,
                    replica_groups=[list(range(N_CORES))],
                    ins=[in_b.opt()], outs=[out_b.opt()],
                )

            # ---- readback + fold + apply ----
            with tc.tile_pool(name="g8", bufs=1) as g8_pool, \
                 tc.tile_pool(name="zb", bufs=2) as zb_pool, \
                 tc.tile_pool(name="mp", bufs=1) as m_pool, \
                 tc.tile_pool(name="fps", bufs=7, space="PSUM") as fps, \
                 tc.tile_pool(name="ups", bufs=1, space="PSUM") as ups, \
                 tc.tile_pool(name="yo", bufs=2) as yo_pool:
                G = {}
                # k = 7 first (the fold consumes it first); alternate queues
                # so trigger issue time does not throttle the fold
                qs = [nc.sync, nc.scalar, nc.gpsimd]
                for k in range(N_CORES - 1, -1, -1):
                    for d in range(ND):
                        g = g8_pool.tile([128, SW], bf16, name=f"G{k}_{d}")
                        qs[(k * ND + d) % 3].dma_start(
                            out=g,
                            in_=out_b[k * D + d * 128:k * D + (d + 1) * 128, :])
                        G[(k, d)] = g

                # ---- fold to (M, c_row) ----
                psu = ups.tile([1, D], f32, name="psu")
                Zcur = None  # k=7 uses the bf16 identity as rhs
                M_tiles = None
                for k in range(N_CORES - 1, -1, -1):
                    lastf = (k == 0)
                    if lastf:
                        Znew = [m_pool.tile([128, D], bf16, name=f"M_{j}")
                                for j in range(ND)]
                        M_tiles = Znew
                    else:
                        Znew = [zb_pool.tile([128, D], bf16,
                                             name=f"Z{k}_{j}", tag=f"Z{j}")
                                for j in range(ND)]

                    def z_ap(d):
                        if Zcur is None:
                            return eye_sb[:, d * D:(d + 1) * D]
                        return Zcur[d]

                    psZ = [fps.tile([128, D], f32, name=f"psZ_{k}_{j}",
                                    tag="psf") for j in range(ND)]
                    # final step: u-matmuls first, so psu stops early and the
                    # c_row/cT bias prep overlaps the last Z-product instead
                    # of stalling the PE at the fold->apply boundary
                    if lastf:
                        for d in range(ND):
                            nc.tensor.matmul(
                                psu, G[(k, d)][:, D:D + 1], z_ap(d),
                                start=False, stop=(d == ND - 1))
                    # d-outer: consume one gathered G tile at a time so the
                    # first fold step overlaps the post-AG readback
                    for d in range(ND):
                        for j in range(ND):
                            nc.tensor.matmul(
                                psZ[j],
                                G[(k, d)][:, j * 128:(j + 1) * 128],
                                z_ap(d),
                                start=(d == 0), stop=(d == ND - 1))
                    for j in range(ND):
                        if j % 2 == 0:
                            nc.vector.tensor_copy(Znew[j], psZ[j])
                        else:
                            nc.scalar.copy(out=Znew[j], in_=psZ[j])
                    if not lastf:
                        for d in range(ND):
                            nc.tensor.matmul(
                                psu,
                                G[(k, d)][:, D:D + 1],
                                z_ap(d),
                                start=(k == N_CORES - 1 and d == 0),
                                stop=False)
                    Zcur = Znew
                c_row = cpool.tile([1, D], bf16, name="c_row")
                nc.vector.tensor_copy(c_row, psu)
                # transpose c_row into per-partition bias columns [128,1]x4
                cT = []
                for j in range(ND):
                    pst = fps.tile([128, 512], f32, name=f"pst_{j}",
                                   tag="psf")
                    nc.tensor.matmul(
                        pst[:, 0:1],
                        c_row[0:1, j * 128:(j + 1) * 128],
                        ones_sb[0:1, 0:1],
                        start=True, stop=True)
                    ct = cpool.tile([128, 1], f32, name=f"cT_{j}")
                    nc.vector.tensor_copy(ct, pst[:, 0:1])
                    cT.append(ct)

                # ---- apply y^T = M^T x^T + c, d-outer ----
                for j in range(ND):
                    psY = [fps.tile([128, 512], f32, name=f"psY_{n}_{j}",
                                    tag="psf") for n in range(NB)]
                    for d in range(ND):
                        for n in range(NB):
                            nc.tensor.matmul(
                                psY[n],
                                M_tiles[d][:, j * 128:(j + 1) * 128],
                                xsb[:, d * B + n * 512:d * B + (n + 1) * 512],
                                start=(d == 0), stop=(d == ND - 1))
                    yo = yo_pool.tile([128, B], f32r, name=f"yo_{j}", tag="yo")
                    for n in range(NB):
                        if n % 2 == 0:
                            nc.vector.tensor_scalar_add(
                                out=yo[:, n * 512:(n + 1) * 512],
                                in0=psY[n], scalar1=cT[j])
                        else:
                            nc.scalar.add(out=yo[:, n * 512:(n + 1) * 512],
                                          in_=psY[n], add=cT[j])
                        nc.sync.dma_start(
                            out=yT[:, j * B + n * 512:j * B + (n + 1) * 512],
                            in_=yo[:, n * 512:(n + 1) * 512])

    nc.compile()
    return nc


def _get_nc():
    key = "default"
    if key not in _NC_CACHE:
        _NC_CACHE[key] = _build_nc()
    return _NC_CACHE[key]


def _segment_bounds():
    bounds = []
    lo = 0
    for ln in SEG_LENS:
        bounds.append((lo, lo + ln))
        lo += ln
    assert lo == N_LAYERS
    return bounds


def _pm(a):
    """[512, X] -> partition-major [128, 4*X] (d-tile blocks side by side)."""
    x = a.shape[1]
    return np.ascontiguousarray(
        a.reshape(ND, 128, x).transpose(1, 0, 2).reshape(128, ND * x))


def kernel(x: np.ndarray, Ws: np.ndarray, bs: np.ndarray) -> np.ndarray:
    global LAST_EXEC_TIME_NS, LAST_RESULTS
    x = np.ascontiguousarray(np.asarray(x, dtype=np.float32))
    Ws = np.ascontiguousarray(np.asarray(Ws, dtype=np.float32))
    bs = np.ascontiguousarray(np.asarray(bs, dtype=np.float32))

    onehot = np.zeros((1, 2), dtype=ml_dtypes.bfloat16)
    onehot[0, 0] = 1.0
    ones_row = np.ones((1, D), dtype=ml_dtypes.bfloat16)
    eye_b = _pm(np.eye(D, dtype=np.float32)).astype(ml_dtypes.bfloat16)

    in_maps = []
    for i, (lo, hi) in enumerate(_segment_bounds()):
        if hi - lo == NSTEPS + 1:
            winit = np.concatenate(
                [Ws[lo], bs[lo][:, None],
                 np.zeros((D, 1), dtype=np.float32)], axis=1)
            steps = list(range(lo + 1, hi))
        else:
            winit = np.concatenate(
                [np.eye(D, dtype=np.float32),
                 np.zeros((D, 2), dtype=np.float32)], axis=1)
            steps = list(range(lo, hi))
        assert len(steps) == NSTEPS
        WTp = np.stack([_pm(np.ascontiguousarray(Ws[l].T)) for l in steps],
                       axis=0).astype(ml_dtypes.bfloat16)
        brow = np.ascontiguousarray(
            np.stack([bs[l] for l in steps], axis=0).reshape(
                1, NSTEPS * D).astype(ml_dtypes.bfloat16))
        shard = _pm(np.ascontiguousarray(
            x[i * B:(i + 1) * B, :].T)).astype(ml_dtypes.bfloat16)
        in_maps.append({
            "xT": shard,
            "WT": np.ascontiguousarray(WTp),
            "Winit": _pm(winit).astype(ml_dtypes.bfloat16),
            "bRow": brow,
            "onehotB": onehot,
            "onesRow": ones_row,
            "eyeB": eye_b,
        })

    nc = _get_nc()
    trace = os.environ.get("BASS_KERNEL_TRACE", "0") == "1"
    res = run_bass_kernel_spmd(nc, in_maps, list(range(N_CORES)), trace=trace)
    LAST_EXEC_TIME_NS = res.exec_time_ns
    LAST_RESULTS = res

    outs = []
    for i in range(N_CORES):
        yp = res.results[i]["yT"]  # [128, 4*2048]: yp[p, j*B+n] = y[n, j*128+p]
        y = yp.reshape(128, ND, B).transpose(2, 1, 0).reshape(B, D)
        outs.append(y)
    y = np.concatenate(outs, axis=0)
    return np.ascontiguousarray(y.astype(np.float32))
